# revision 9
# baseline (speedup 1.0000x reference)
"""Trainium2 Bass kernel for AffineMultiQueryHardAttentionEncoder.

reference:
    scores = max_n( (queries * affine) @ keys.T )        # [M]
    w, idx = top_k(scores, 64); w = softmax(w)
    encoding = sum(w[:, None] * values[idx], 0)          # [D]
    returns (encoding, idx)

Strategy (8 NeuronCores):
  Launch 1 (SPMD x8): shard keys along M (12500/core, padded to 12800).
    Each core: bf16 matmul scan  scores[m] = max_n (q*a).k_m  with m on
    partitions / n on the free dim, DVE reduce_max over n, then per-partition
    top-16 candidates via the DVE max8/match_replace/max_index primitives.
    -> 2048 candidate (value, index) pairs per core.
  Host: merge 8*2048 candidates, keep global top-128 by (bf16-noisy) score,
    gather those keys/values rows (data movement only).
  Launch 2 (1 core): recompute the 128 candidate scores exactly in fp32,
    exact top-64 selection by rank (all-pairs compare), index recovery,
    softmax, weighted sum of value rows — all on device.

The bf16 scan only needs to rank well enough that the true top-64 are inside
the noisy top-128 of the union of per-partition top-16s. Measured on the
fixed inputs: bf16 score noise <= 0.32 abs; worst noisy global rank of a
true-top-64 element is 66 (margin 128), worst per-partition rank is 1
(margin 16). The fp32 recompute restores exact ordering and weights.
"""

import os
import numpy as np
import ml_dtypes

import concourse.bass as bass
import concourse.bacc as bacc
import concourse.mybir as mybir
from concourse.tile import TileContext
from concourse.bass_utils import run_bass_kernel_spmd

BF16 = ml_dtypes.bfloat16
F32 = mybir.dt.float32
U32 = mybir.dt.uint32

N_CORES = 8
N = 512          # queries
M = 100000       # keys/values rows
D = 1024         # feature dim
KTOP = 64
DCH = D // 128   # 8 contraction chunks

M_SHARD = M // N_CORES          # 12500
M_PAD = 12800                   # per-core padded key count
# first swaths small so the PE can start while queries/keys stream in
SWATHS = [512, 768] + [1280] * 9
assert sum(SWATHS) == M_PAD
NT = M_PAD // 128               # 100 m-tiles -> scores [128, NT]

CAND_ROUNDS = 2                 # per-partition top-(8*rounds)
CAND_PER_P = 8 * CAND_ROUNDS    # 16
MARGIN = 128                    # global noisy-candidate count re-scored exactly

NEG = -1.0e30

_cache: dict = {}


def _build_scan(m_pad=M_PAD, swaths=None):
    """Launch-1 program: bf16 score scan + per-partition top-16 candidates."""
    if swaths is None:
        swaths = SWATHS if m_pad == M_PAD else [1280] * (m_pad // 1280)
    assert sum(swaths) == m_pad
    nt = m_pad // 128
    max_sw = max(swaths)

    nc = bacc.Bacc("TRN2", debug=False)
    kT = nc.dram_tensor("keysT", [D, m_pad], mybir.dt.bfloat16, kind="ExternalInput")
    qT = nc.dram_tensor("queriesT", [D, N], mybir.dt.bfloat16, kind="ExternalInput")
    aff = nc.dram_tensor("affpc", [128, DCH], F32, kind="ExternalInput")
    o_vals = nc.dram_tensor("cand_vals", [128, CAND_PER_P], F32, kind="ExternalOutput")
    o_idx = nc.dram_tensor("cand_idx", [128, CAND_PER_P], U32, kind="ExternalOutput")
    o_warm = nc.dram_tensor("warm", [128, 1], F32, kind="ExternalOutput")

    with TileContext(nc) as tc:
        with (
            tc.tile_pool(name="const", bufs=1) as cpool,
            tc.tile_pool(name="keys", bufs=2) as kpool,
            tc.tile_pool(name="ps", bufs=4, space="PSUM") as ppool,
            tc.tile_pool(name="wk", bufs=1) as wpool,
        ):
            # HAM warm-up: dead-weight matmuls with no input deps keep the PE
            # busy during the initial DMA so real matmuls run at 2.4 GHz.
            wg = cpool.tile([128, N], mybir.dt.bfloat16)
            nc.gpsimd.memset(wg, 0.0)
            psw = ppool.tile([128, N], F32, tag="warm")
            for i in range(10):
                nc.tensor.matmul(psw[:], lhsT=wg[:, :128], rhs=wg[:],
                                 start=(i == 0), stop=(i == 9))
            warm = wpool.tile([128, 1], F32)
            nc.vector.reduce_max(out=warm, in_=psw[:], axis=mybir.AxisListType.X)
            nc.sync.dma_start(out=o_warm.ap(), in_=warm)

            afft = cpool.tile([128, DCH], F32)
            nc.sync.dma_start(out=afft, in_=aff.ap())

            # first swath's keys interleaved with the query chunks so the
            # d-accumulation pipeline starts as soon as chunk 0 lands
            s0 = swaths[0]
            kt0 = [kpool.tile([128, max_sw], mybir.dt.bfloat16,
                              name=f"kt{d}", tag=f"k{d}") for d in range(DCH)]
            qf = cpool.tile([128, DCH * N], mybir.dt.bfloat16)
            qb = cpool.tile([128, DCH * N], mybir.dt.bfloat16)
            for d in range(DCH):
                nc.sync.dma_start(out=kt0[d][:, :s0],
                                  in_=kT.ap()[d * 128:(d + 1) * 128, 0:s0])
                nc.sync.dma_start(out=qf[:, d * N:(d + 1) * N],
                                  in_=qT.ap()[d * 128:(d + 1) * 128, :])
                nc.vector.tensor_scalar_mul(
                    qb[:, d * N:(d + 1) * N],
                    qf[:, d * N:(d + 1) * N],
                    afft[:, d:d + 1],
                )

            scores = wpool.tile([128, nt], F32)
            t = 0
            m_off = 0
            for s, sw in enumerate(swaths):
                if s == 0:
                    kt = kt0
                else:
                    kt = [kpool.tile([128, max_sw], mybir.dt.bfloat16,
                                     name=f"kt{d}", tag=f"k{d}") for d in range(DCH)]
                    for d in range(DCH):
                        nc.sync.dma_start(
                            out=kt[d][:, :sw],
                            in_=kT.ap()[d * 128:(d + 1) * 128, m_off:m_off + sw],
                        )
                for mt in range(sw // 128):
                    ps = ppool.tile([128, N], F32, tag="mm")
                    for d in range(DCH):
                        nc.tensor.matmul(
                            ps[:],
                            lhsT=kt[d][:, mt * 128:(mt + 1) * 128],
                            rhs=qb[:, d * N:(d + 1) * N],
                            start=(d == 0),
                            stop=(d == DCH - 1),
                        )
                    nc.vector.reduce_max(
                        out=scores[:, t:t + 1], in_=ps[:], axis=mybir.AxisListType.X
                    )
                    t += 1
                m_off += sw

            # Padded key columns are zero -> score exactly 0, while every real
            # score is the max of 512 ~N(0, 32) dots (>= +60 in practice), so
            # pads can never enter a per-partition top-16. The host also
            # filters m >= M as a belt-and-braces guard.
            cv = wpool.tile([128, CAND_PER_P], F32)
            ci = wpool.tile([128, CAND_PER_P], U32)
            for r in range(CAND_ROUNDS):
                sl = slice(r * 8, (r + 1) * 8)
                nc.vector.max(out=cv[:, sl], in_=scores[:])
                nc.vector.max_index(out=ci[:, sl], in_max=cv[:, sl], in_values=scores[:])
                if r < CAND_ROUNDS - 1:
                    nc.vector.match_replace(
                        out=scores[:], in_to_replace=cv[:, sl],
                        in_values=scores[:], imm_value=NEG,
                    )
            nc.sync.dma_start(out=o_vals.ap(), in_=cv)
            nc.sync.dma_start(out=o_idx.ap(), in_=ci)
    nc.compile()
    return nc


def _build_combine():
    """Launch-2 program (single core): exact fp32 re-score of MARGIN candidates,
    exact top-64 via all-pairs rank select, softmax, weighted sum of values."""
    nc = bacc.Bacc("TRN2", debug=False)
    kTc = nc.dram_tensor("kTc", [D, MARGIN], F32, kind="ExternalInput")
    vc = nc.dram_tensor("vc", [MARGIN, D], F32, kind="ExternalInput")
    mi = nc.dram_tensor("midx", [MARGIN, 1], F32, kind="ExternalInput")
    qT = nc.dram_tensor("queriesT", [D, N], F32, kind="ExternalInput")
    aff = nc.dram_tensor("affpc", [128, DCH], F32, kind="ExternalInput")
    iden = nc.dram_tensor("iden", [128, 128], F32, kind="ExternalInput")
    jrow = nc.dram_tensor("jrow", [128, KTOP], F32, kind="ExternalInput")
    o_enc = nc.dram_tensor("enc", [1, D], F32, kind="ExternalOutput")
    o_ind = nc.dram_tensor("ind", [1, KTOP], F32, kind="ExternalOutput")
    o_top = nc.dram_tensor("topvals", [1, KTOP], F32, kind="ExternalOutput")
    o_warm = nc.dram_tensor("warm", [128, 1], F32, kind="ExternalOutput")

    with TileContext(nc) as tc:
        with (
            tc.tile_pool(name="c2", bufs=1) as cpool,
            tc.tile_pool(name="p2", bufs=1, space="PSUM") as ppool,
        ):
            # HAM warm-up during input DMA
            wg = cpool.tile([128, N], F32)
            nc.gpsimd.memset(wg, 0.0)
            psw = ppool.tile([128, N], F32, tag="warm")
            for i in range(6):
                nc.tensor.matmul(psw[:], lhsT=wg[:, :128], rhs=wg[:],
                                 start=(i == 0), stop=(i == 5))
            warm = cpool.tile([128, 1], F32)
            nc.vector.reduce_max(out=warm, in_=psw[:], axis=mybir.AxisListType.X)
            nc.sync.dma_start(out=o_warm.ap(), in_=warm)

            afft = cpool.tile([128, DCH], F32)
            nc.sync.dma_start(out=afft, in_=aff.ap())
            qf = cpool.tile([128, DCH * N], F32)
            qa = cpool.tile([128, DCH * N], F32)
            ktile = cpool.tile([128, DCH * MARGIN], F32)
            for d in range(DCH):
                nc.sync.dma_start(
                    out=ktile[:, d * MARGIN:(d + 1) * MARGIN],
                    in_=kTc.ap()[d * 128:(d + 1) * 128, :],
                )
                nc.sync.dma_start(
                    out=qf[:, d * N:(d + 1) * N],
                    in_=qT.ap()[d * 128:(d + 1) * 128, :],
                )
                nc.vector.tensor_scalar_mul(
                    qa[:, d * N:(d + 1) * N],
                    qf[:, d * N:(d + 1) * N],
                    afft[:, d:d + 1],
                )
            idt = cpool.tile([128, 128], F32)
            nc.sync.dma_start(out=idt, in_=iden.ap())
            mit = cpool.tile([128, 1], F32)
            nc.sync.dma_start(out=mit, in_=mi.ap())
            jrt = cpool.tile([128, KTOP], F32)
            nc.sync.dma_start(out=jrt, in_=jrow.ap())
            vt = cpool.tile([128, D], F32)
            nc.sync.dma_start(out=vt, in_=vc.ap())

            # exact fp32 scores of the MARGIN candidates
            ps = ppool.tile([128, N], F32, tag="score")
            for d in range(DCH):
                nc.tensor.matmul(
                    ps[:],
                    lhsT=ktile[:, d * MARGIN:(d + 1) * MARGIN],
                    rhs=qa[:, d * N:(d + 1) * N],
                    start=(d == 0),
                    stop=(d == DCH - 1),
                )
            sc = cpool.tile([128, 1], F32)
            nc.vector.reduce_max(out=sc, in_=ps[:], axis=mybir.AxisListType.X)

            # transpose scores [128,1] -> [1,128] via identity matmul
            pst = ppool.tile([1, 128], F32, tag="tr")
            nc.tensor.matmul(pst[:], lhsT=sc[:], rhs=idt[:], start=True, stop=True)
            scr = cpool.tile([1, 128], F32)
            nc.vector.tensor_copy(out=scr, in_=pst[:])

            # softmax max = global max (top-1 of the row)
            mx8 = cpool.tile([1, 8], F32)
            nc.vector.max(out=mx8, in_=scr[:])

            # broadcast the score row to all partitions; rank by all-pairs compare
            ones_r = cpool.tile([1, 128], F32)
            nc.vector.memset(ones_r, 1.0)
            psb = ppool.tile([128, 128], F32, tag="bcast")
            nc.tensor.matmul(psb[:], lhsT=ones_r[:], rhs=scr[:], start=True, stop=True)
            scb = cpool.tile([128, 128], F32)
            nc.vector.tensor_copy(out=scb, in_=psb[:])
            gt = cpool.tile([128, 128], F32)
            nc.vector.tensor_tensor(
                gt[:], scb[:], sc.to_broadcast([128, 128]), mybir.AluOpType.is_gt
            )
            rank = cpool.tile([128, 1], F32)
            nc.vector.reduce_sum(out=rank, in_=gt[:], axis=mybir.AxisListType.X)

            # selection mask (rank < 64) and rank->slot equality matrix
            sel = cpool.tile([128, 1], F32)
            nc.vector.tensor_scalar(sel, rank, float(KTOP), None,
                                    op0=mybir.AluOpType.is_lt)
            eqj = cpool.tile([128, KTOP], F32)
            nc.vector.tensor_tensor(
                eqj[:], rank.to_broadcast([128, KTOP]), jrt[:], mybir.AluOpType.is_equal
            )

            # sorted indices and values: pair[:, 0:64] = eqj * m_idx,
            # pair[:, 64:128] = eqj * score; one partition-sum matmul for both
            pair = cpool.tile([128, 2 * KTOP], F32)
            nc.vector.tensor_scalar_mul(pair[:, 0:KTOP], eqj[:], mit[:, 0:1])
            nc.vector.tensor_scalar_mul(pair[:, KTOP:2 * KTOP], eqj[:], sc[:, 0:1])
            ones_c = cpool.tile([128, 1], F32)
            nc.vector.memset(ones_c, 1.0)
            pp = ppool.tile([1, 2 * KTOP], F32, tag="iv")
            nc.tensor.matmul(pp[:], lhsT=ones_c[:], rhs=pair[:], start=True, stop=True)
            indvals = cpool.tile([1, 2 * KTOP], F32)
            nc.vector.tensor_copy(out=indvals, in_=pp[:])
            nc.sync.dma_start(out=o_ind.ap(), in_=indvals[:, 0:KTOP])
            nc.sync.dma_start(out=o_top.ap(), in_=indvals[:, KTOP:2 * KTOP])

            # softmax numerators on the candidate axis (masked), Z via matmul
            nm0 = cpool.tile([1, 1], F32)
            nc.vector.tensor_scalar_mul(nm0, mx8[:, 0:1], -1.0)
            pnb = ppool.tile([128, 1], F32, tag="nm")
            nc.tensor.matmul(pnb[:], lhsT=ones_r[:], rhs=nm0[:], start=True, stop=True)
            nm0b = cpool.tile([128, 1], F32)
            nc.vector.tensor_copy(out=nm0b, in_=pnb[:])
            ex = cpool.tile([128, 1], F32)
            nc.scalar.activation(out=ex[:], in_=sc[:],
                                 func=mybir.ActivationFunctionType.Exp,
                                 bias=nm0b[:, 0:1], scale=1.0)
            esel = cpool.tile([128, 1], F32)
            nc.vector.tensor_tensor(esel[:], ex[:], sel[:], mybir.AluOpType.mult)

            # unnormalized encoding and Z, then scale by 1/Z
            pz = ppool.tile([1, 1], F32, tag="z")
            nc.tensor.matmul(pz[:], lhsT=esel[:], rhs=ones_c[:], start=True, stop=True)
            zsb = cpool.tile([1, 1], F32)
            nc.vector.tensor_copy(out=zsb, in_=pz[:])
            rz = cpool.tile([1, 1], F32)
            nc.vector.reciprocal(out=rz, in_=zsb[:])

            enc = cpool.tile([1, D], F32)
            for h in range(2):
                pse = ppool.tile([1, 512], F32, tag="enc")
                nc.tensor.matmul(
                    pse[:], lhsT=esel[:], rhs=vt[:, h * 512:(h + 1) * 512],
                    start=True, stop=True,
                )
                nc.vector.tensor_copy(out=enc[:, h * 512:(h + 1) * 512], in_=pse[:])
            nc.vector.tensor_scalar_mul(enc[:], enc[:], rz[:, 0:1])
            nc.sync.dma_start(out=o_enc.ap(), in_=enc)
    nc.compile()
    return nc


def _get_programs():
    if "scan" not in _cache:
        _cache["scan"] = _build_scan()
    if "combine" not in _cache:
        _cache["combine"] = _build_combine()
    return _cache["scan"], _cache["combine"]


def kernel(queries, keys, values, affine):
    queries = np.asarray(queries, dtype=np.float32)
    keys = np.asarray(keys, dtype=np.float32)
    values = np.asarray(values, dtype=np.float32)
    affine = np.asarray(affine, dtype=np.float32)

    trace = bool(int(os.environ.get("KERNEL_TRACE", "0")))
    nc1, nc2 = _get_programs()

    qT = np.ascontiguousarray(queries.T)                 # [D, N] f32
    qT_bf = qT.astype(BF16)
    affpc = np.ascontiguousarray(affine.reshape(DCH, 128).T)  # [128, DCH]
    kT_all = keys.T.astype(BF16)                         # [D, M]

    in_maps = []
    for c in range(N_CORES):
        kc = np.zeros((D, M_PAD), dtype=BF16)
        kc[:, :M_SHARD] = kT_all[:, c * M_SHARD:(c + 1) * M_SHARD]
        in_maps.append({"keysT": kc, "queriesT": qT_bf, "affpc": affpc})

    r1 = run_bass_kernel_spmd(nc1, in_maps, list(range(N_CORES)), trace=trace)
    _cache["last_scan_results"] = r1

    vals = np.stack([r1.results[c]["cand_vals"] for c in range(N_CORES)])  # [8,128,16]
    idxt = np.stack([r1.results[c]["cand_idx"] for c in range(N_CORES)])   # [8,128,16]
    p_arr = np.arange(128, dtype=np.int64)[None, :, None]
    c_arr = np.arange(N_CORES, dtype=np.int64)[:, None, None]
    m_glob = idxt.astype(np.int64) * 128 + p_arr + c_arr * M_SHARD
    vals_f = vals.reshape(-1)
    m_f = m_glob.reshape(-1)
    ok = m_f < M                      # paranoia; padding never ranks anyway
    vals_f, m_f = vals_f[ok], m_f[ok]
    sel = np.argsort(-vals_f, kind="stable")[:MARGIN]
    m_sel = m_f[sel]

    kTc = np.ascontiguousarray(keys[m_sel].T)            # [D, MARGIN]
    vcand = np.ascontiguousarray(values[m_sel])          # [MARGIN, D]
    midx = m_sel.astype(np.float32)[:, None]             # [MARGIN, 1]
    iden = np.eye(128, dtype=np.float32)
    jrow = np.tile(np.arange(KTOP, dtype=np.float32), (128, 1))

    in2 = {"kTc": kTc, "vc": vcand, "midx": midx,
           "queriesT": qT, "affpc": affpc, "iden": iden, "jrow": jrow}
    r2 = run_bass_kernel_spmd(nc2, [in2], [0], trace=trace)
    _cache["last_combine_results"] = r2

    encoding = np.asarray(r2.results[0]["enc"][0], dtype=np.float32)
    indices = np.rint(np.asarray(r2.results[0]["ind"][0])).astype(np.int32)
    return encoding, indices


# revision 18
# speedup vs baseline: 1.0273x; 1.0273x over previous
"""Trainium2 Bass kernel for AffineMultiQueryHardAttentionEncoder.

reference:
    scores = max_n( (queries * affine) @ keys.T )        # [M]
    w, idx = top_k(scores, 64); w = softmax(w)
    encoding = sum(w[:, None] * values[idx], 0)          # [D]
    returns (encoding, idx)

Strategy (8 NeuronCores):
  Launch 1 (SPMD x8): shard keys along M (12500/core, padded to 12800).
    Each core: bf16 matmul scan  scores[m] = max_n (q*a).k_m  with m on
    partitions / n on the free dim, DVE reduce_max over n, then per-partition
    top-16 candidates via the DVE max8/match_replace/max_index primitives.
    -> 2048 candidate (value, index) pairs per core.
  Host: merge 8*2048 candidates, keep global top-128 by (bf16-noisy) score,
    gather those keys/values rows (data movement only).
  Launch 2 (1 core): recompute the 128 candidate scores exactly in fp32,
    exact top-64 selection by rank (all-pairs compare), index recovery,
    softmax, weighted sum of value rows — all on device.

The bf16 scan only needs to rank well enough that the true top-64 are inside
the noisy top-128 of the union of per-partition top-16s. Measured on the
fixed inputs: bf16 score noise <= 0.32 abs; worst noisy global rank of a
true-top-64 element is 66 (margin 128), worst per-partition rank is 1
(margin 16). The fp32 recompute restores exact ordering and weights.
"""

import os
import numpy as np
import ml_dtypes

import concourse.bass as bass
import concourse.bacc as bacc
import concourse.mybir as mybir
from concourse.tile import TileContext
from concourse.bass_utils import run_bass_kernel_spmd

BF16 = ml_dtypes.bfloat16
F32 = mybir.dt.float32
U32 = mybir.dt.uint32

N_CORES = 8
N = 512          # queries
M = 100000       # keys/values rows
D = 1024         # feature dim
KTOP = 64
DCH = D // 128   # 8 contraction chunks

M_SHARD = M // N_CORES          # 12500
M_PAD = 12800                   # per-core padded key count
# first swaths small so the PE can start while queries/keys stream in
SWATHS = [512, 768] + [1280] * 9
assert sum(SWATHS) == M_PAD
NT = M_PAD // 128               # 100 m-tiles -> scores [128, NT]

CAND_ROUNDS = 2                 # per-partition top-(8*rounds)
CAND_PER_P = 8 * CAND_ROUNDS    # 16
MARGIN = 128                    # global noisy-candidate count re-scored exactly

NEG = -1.0e30

_cache: dict = {}


def _build_scan(m_pad=M_PAD, swaths=None):
    """Launch-1 program: bf16 score scan + per-partition top-16 candidates."""
    if swaths is None:
        swaths = SWATHS if m_pad == M_PAD else [1280] * (m_pad // 1280)
    assert sum(swaths) == m_pad
    nt = m_pad // 128
    max_sw = max(swaths)

    nc = bacc.Bacc("TRN2", debug=False)
    kT = nc.dram_tensor("keysT", [D, m_pad], mybir.dt.bfloat16, kind="ExternalInput")
    qT = nc.dram_tensor("queriesT", [D, N], mybir.dt.bfloat16, kind="ExternalInput")
    aff = nc.dram_tensor("affpc", [128, DCH], F32, kind="ExternalInput")
    o_vals = nc.dram_tensor("cand_vals", [128, CAND_PER_P], F32, kind="ExternalOutput")
    o_idx = nc.dram_tensor("cand_idx", [128, CAND_PER_P], U32, kind="ExternalOutput")

    with TileContext(nc) as tc:
        with (
            tc.tile_pool(name="const", bufs=1) as cpool,
            tc.tile_pool(name="keys", bufs=2) as kpool,
            tc.tile_pool(name="ps", bufs=4, space="PSUM") as ppool,
            tc.tile_pool(name="wk", bufs=1) as wpool,
        ):
            # queries + affine arrive via the gpsimd (SWDGE) queue so the
            # sync (HWDGE) queue is dedicated to streaming keys
            afft = cpool.tile([128, DCH], F32)
            nc.gpsimd.dma_start(out=afft, in_=aff.ap())
            qf = cpool.tile([128, DCH * N], mybir.dt.bfloat16)
            nc.gpsimd.dma_start(
                out=qf[:].rearrange("p (d n) -> p d n", n=N),
                in_=qT.ap().rearrange("(d p) n -> p d n", p=128))
            qb = cpool.tile([128, DCH * N], mybir.dt.bfloat16)
            for d in range(DCH):
                nc.vector.tensor_scalar_mul(
                    qb[:, d * N:(d + 1) * N],
                    qf[:, d * N:(d + 1) * N],
                    afft[:, d:d + 1],
                )

            scores = wpool.tile([128, nt], F32)
            t = 0
            m_off = 0
            for s, sw in enumerate(swaths):
                # one DMA per swath: all 8 d-chunks side by side
                kt = kpool.tile([128, DCH * max_sw], mybir.dt.bfloat16,
                                name="kt", tag="k")
                nc.sync.dma_start(
                    out=kt[:, :DCH * sw].rearrange("p (d m) -> p d m", m=sw),
                    in_=kT.ap()[:, m_off:m_off + sw].rearrange(
                        "(d p) m -> p d m", p=128),
                )
                for mt in range(sw // 128):
                    ps = ppool.tile([128, N], F32, tag="mm")
                    for d in range(DCH):
                        nc.tensor.matmul(
                            ps[:],
                            lhsT=kt[:, d * sw + mt * 128:d * sw + (mt + 1) * 128],
                            rhs=qb[:, d * N:(d + 1) * N],
                            start=(d == 0),
                            stop=(d == DCH - 1),
                        )
                    nc.vector.reduce_max(
                        out=scores[:, t:t + 1], in_=ps[:], axis=mybir.AxisListType.X
                    )
                    t += 1
                m_off += sw

            # Padded key columns are zero -> score exactly 0, while every real
            # score is the max of 512 ~N(0, 32) dots (>= +60 in practice), so
            # pads can never enter a per-partition top-16. The host also
            # filters m >= M as a belt-and-braces guard.
            cv = wpool.tile([128, CAND_PER_P], F32)
            ci = wpool.tile([128, CAND_PER_P], U32)
            for r in range(CAND_ROUNDS):
                sl = slice(r * 8, (r + 1) * 8)
                nc.vector.max(out=cv[:, sl], in_=scores[:])
                nc.vector.max_index(out=ci[:, sl], in_max=cv[:, sl], in_values=scores[:])
                if r < CAND_ROUNDS - 1:
                    nc.vector.match_replace(
                        out=scores[:], in_to_replace=cv[:, sl],
                        in_values=scores[:], imm_value=NEG,
                    )
            nc.sync.dma_start(out=o_vals.ap(), in_=cv)
            nc.sync.dma_start(out=o_idx.ap(), in_=ci)
    nc.compile()
    return nc


def _build_combine():
    """Launch-2 program (single core): exact fp32 re-score of MARGIN candidates,
    exact top-64 via all-pairs rank select, softmax, weighted sum of values."""
    nc = bacc.Bacc("TRN2", debug=False)
    kTc = nc.dram_tensor("kTc", [D, MARGIN], F32, kind="ExternalInput")
    vc = nc.dram_tensor("vc", [MARGIN, D], F32, kind="ExternalInput")
    mi = nc.dram_tensor("midx", [MARGIN, 1], F32, kind="ExternalInput")
    qT = nc.dram_tensor("queriesT", [D, N], F32, kind="ExternalInput")
    aff = nc.dram_tensor("affpc", [128, DCH], F32, kind="ExternalInput")
    iden = nc.dram_tensor("iden", [128, 128], F32, kind="ExternalInput")
    jrow = nc.dram_tensor("jrow", [128, KTOP], F32, kind="ExternalInput")
    o_enc = nc.dram_tensor("enc", [1, D], F32, kind="ExternalOutput")
    o_ind = nc.dram_tensor("ind", [1, KTOP], F32, kind="ExternalOutput")
    o_top = nc.dram_tensor("topvals", [1, KTOP], F32, kind="ExternalOutput")
    o_warm = nc.dram_tensor("warm", [128, 1], F32, kind="ExternalOutput")

    with TileContext(nc) as tc:
        with (
            tc.tile_pool(name="c2", bufs=1) as cpool,
            tc.tile_pool(name="p2", bufs=1, space="PSUM") as ppool,
        ):
            # HAM warm-up during input DMA
            wg = cpool.tile([128, N], F32)
            nc.vector.memset(wg, 0.0)
            psw = ppool.tile([128, N], F32, tag="warm")
            for i in range(4):
                nc.tensor.matmul(psw[:], lhsT=wg[:, :128], rhs=wg[:],
                                 start=(i == 0), stop=(i == 3))
            warm = cpool.tile([128, 1], F32)
            nc.vector.reduce_max(out=warm, in_=psw[:], axis=mybir.AxisListType.X)
            nc.gpsimd.dma_start(out=o_warm.ap(), in_=warm)

            # big streams on the sync (HWDGE) queue, small constants on the
            # gpsimd (SWDGE) queue -- one DMA per tensor (issue is ~0.6us each)
            ktile = cpool.tile([128, DCH * MARGIN], F32)
            nc.sync.dma_start(
                out=ktile[:].rearrange("p (d m) -> p d m", m=MARGIN),
                in_=kTc.ap().rearrange("(d p) m -> p d m", p=128))
            qf = cpool.tile([128, DCH * N], F32)
            nc.sync.dma_start(
                out=qf[:].rearrange("p (d n) -> p d n", n=N),
                in_=qT.ap().rearrange("(d p) n -> p d n", p=128))
            vt = cpool.tile([128, D], F32)
            nc.sync.dma_start(out=vt, in_=vc.ap())
            afft = cpool.tile([128, DCH], F32)
            nc.gpsimd.dma_start(out=afft, in_=aff.ap())
            idt = cpool.tile([128, 128], F32)
            nc.gpsimd.dma_start(out=idt, in_=iden.ap())
            mit = cpool.tile([128, 1], F32)
            nc.gpsimd.dma_start(out=mit, in_=mi.ap())
            jrt = cpool.tile([128, KTOP], F32)
            nc.gpsimd.dma_start(out=jrt, in_=jrow.ap())

            qa = cpool.tile([128, DCH * N], F32)
            for d in range(DCH):
                nc.vector.tensor_scalar_mul(
                    qa[:, d * N:(d + 1) * N],
                    qf[:, d * N:(d + 1) * N],
                    afft[:, d:d + 1],
                )

            # exact fp32 scores of the MARGIN candidates
            ps = ppool.tile([128, N], F32, tag="score")
            for d in range(DCH):
                nc.tensor.matmul(
                    ps[:],
                    lhsT=ktile[:, d * MARGIN:(d + 1) * MARGIN],
                    rhs=qa[:, d * N:(d + 1) * N],
                    start=(d == 0),
                    stop=(d == DCH - 1),
                )
            sc = cpool.tile([128, 1], F32)
            nc.vector.reduce_max(out=sc, in_=ps[:], axis=mybir.AxisListType.X)

            # transpose scores [128,1] -> [1,128] via identity matmul
            pst = ppool.tile([1, 128], F32, tag="tr")
            nc.tensor.matmul(pst[:], lhsT=sc[:], rhs=idt[:], start=True, stop=True)
            scr = cpool.tile([1, 128], F32)
            nc.vector.tensor_copy(out=scr, in_=pst[:])

            # softmax max = global max (top-1 of the row)
            mx8 = cpool.tile([1, 8], F32)
            nc.vector.max(out=mx8, in_=scr[:])

            # broadcast the score row to all partitions; rank by all-pairs compare
            ones_r = cpool.tile([1, 128], F32)
            nc.vector.memset(ones_r, 1.0)
            psb = ppool.tile([128, 128], F32, tag="bcast")
            nc.tensor.matmul(psb[:], lhsT=ones_r[:], rhs=scr[:], start=True, stop=True)
            gt = cpool.tile([128, 128], F32)
            nc.vector.tensor_tensor(
                gt[:], psb[:], sc.to_broadcast([128, 128]), mybir.AluOpType.is_gt
            )
            rank = cpool.tile([128, 1], F32)
            nc.vector.reduce_sum(out=rank, in_=gt[:], axis=mybir.AxisListType.X)

            # selection mask (rank < 64) and rank->slot equality matrix
            sel = cpool.tile([128, 1], F32)
            nc.vector.tensor_scalar(sel, rank, float(KTOP), None,
                                    op0=mybir.AluOpType.is_lt)
            eqj = cpool.tile([128, KTOP], F32)
            nc.vector.tensor_tensor(
                eqj[:], rank.to_broadcast([128, KTOP]), jrt[:], mybir.AluOpType.is_equal
            )

            # sorted indices and values: pair[:, 0:64] = eqj * m_idx,
            # pair[:, 64:128] = eqj * score; one partition-sum matmul for both
            pair = cpool.tile([128, 2 * KTOP], F32)
            nc.vector.tensor_scalar_mul(pair[:, 0:KTOP], eqj[:], mit[:, 0:1])
            nc.vector.tensor_scalar_mul(pair[:, KTOP:2 * KTOP], eqj[:], sc[:, 0:1])
            ones_c = cpool.tile([128, 1], F32)
            nc.vector.memset(ones_c, 1.0)
            pp = ppool.tile([1, 2 * KTOP], F32, tag="iv")
            nc.tensor.matmul(pp[:], lhsT=ones_c[:], rhs=pair[:], start=True, stop=True)
            indvals = cpool.tile([1, 2 * KTOP], F32)
            nc.vector.tensor_copy(out=indvals, in_=pp[:])
            nc.gpsimd.dma_start(out=o_ind.ap(), in_=indvals[:, 0:KTOP])
            nc.gpsimd.dma_start(out=o_top.ap(), in_=indvals[:, KTOP:2 * KTOP])

            # softmax numerators on the candidate axis (masked), Z via matmul
            nm0 = cpool.tile([1, 1], F32)
            nc.vector.tensor_scalar_mul(nm0, mx8[:, 0:1], -1.0)
            pnb = ppool.tile([128, 1], F32, tag="nm")
            nc.tensor.matmul(pnb[:], lhsT=ones_r[:], rhs=nm0[:], start=True, stop=True)
            nm0b = cpool.tile([128, 1], F32)
            nc.vector.tensor_copy(out=nm0b, in_=pnb[:])
            ex = cpool.tile([128, 1], F32)
            nc.scalar.activation(out=ex[:], in_=sc[:],
                                 func=mybir.ActivationFunctionType.Exp,
                                 bias=nm0b[:, 0:1], scale=1.0)
            esel = cpool.tile([128, 1], F32)
            nc.vector.tensor_tensor(esel[:], ex[:], sel[:], mybir.AluOpType.mult)

            # unnormalized encoding and Z, then scale by 1/Z while copying out
            pz = ppool.tile([1, 1], F32, tag="z")
            nc.tensor.matmul(pz[:], lhsT=esel[:], rhs=ones_c[:], start=True, stop=True)
            rz = cpool.tile([1, 1], F32)
            nc.vector.reciprocal(out=rz, in_=pz[:])

            enc = cpool.tile([1, D], F32)
            for h in range(2):
                pse = ppool.tile([1, 512], F32, tag="enc")
                nc.tensor.matmul(
                    pse[:], lhsT=esel[:], rhs=vt[:, h * 512:(h + 1) * 512],
                    start=True, stop=True,
                )
                nc.vector.tensor_scalar_mul(
                    enc[:, h * 512:(h + 1) * 512], pse[:], rz[:, 0:1])
            nc.sync.dma_start(out=o_enc.ap(), in_=enc)
    nc.compile()
    return nc


def _get_programs():
    if "scan" not in _cache:
        _cache["scan"] = _build_scan()
    if "combine" not in _cache:
        _cache["combine"] = _build_combine()
    return _cache["scan"], _cache["combine"]


def kernel(queries, keys, values, affine):
    queries = np.asarray(queries, dtype=np.float32)
    keys = np.asarray(keys, dtype=np.float32)
    values = np.asarray(values, dtype=np.float32)
    affine = np.asarray(affine, dtype=np.float32)

    trace = bool(int(os.environ.get("KERNEL_TRACE", "0")))
    nc1, nc2 = _get_programs()

    qT = np.ascontiguousarray(queries.T)                 # [D, N] f32
    qT_bf = qT.astype(BF16)
    affpc = np.ascontiguousarray(affine.reshape(DCH, 128).T)  # [128, DCH]
    kT_all = keys.T.astype(BF16)                         # [D, M]

    in_maps = []
    for c in range(N_CORES):
        kc = np.zeros((D, M_PAD), dtype=BF16)
        kc[:, :M_SHARD] = kT_all[:, c * M_SHARD:(c + 1) * M_SHARD]
        in_maps.append({"keysT": kc, "queriesT": qT_bf, "affpc": affpc})

    r1 = run_bass_kernel_spmd(nc1, in_maps, list(range(N_CORES)), trace=trace)
    _cache["last_scan_results"] = r1

    vals = np.stack([r1.results[c]["cand_vals"] for c in range(N_CORES)])  # [8,128,16]
    idxt = np.stack([r1.results[c]["cand_idx"] for c in range(N_CORES)])   # [8,128,16]
    p_arr = np.arange(128, dtype=np.int64)[None, :, None]
    c_arr = np.arange(N_CORES, dtype=np.int64)[:, None, None]
    m_glob = idxt.astype(np.int64) * 128 + p_arr + c_arr * M_SHARD
    vals_f = vals.reshape(-1)
    m_f = m_glob.reshape(-1)
    ok = m_f < M                      # paranoia; padding never ranks anyway
    vals_f, m_f = vals_f[ok], m_f[ok]
    sel = np.argsort(-vals_f, kind="stable")[:MARGIN]
    m_sel = m_f[sel]

    kTc = np.ascontiguousarray(keys[m_sel].T)            # [D, MARGIN]
    vcand = np.ascontiguousarray(values[m_sel])          # [MARGIN, D]
    midx = m_sel.astype(np.float32)[:, None]             # [MARGIN, 1]
    iden = np.eye(128, dtype=np.float32)
    jrow = np.tile(np.arange(KTOP, dtype=np.float32), (128, 1))

    in2 = {"kTc": kTc, "vc": vcand, "midx": midx,
           "queriesT": qT, "affpc": affpc, "iden": iden, "jrow": jrow}
    r2 = run_bass_kernel_spmd(nc2, [in2], [0], trace=trace)
    _cache["last_combine_results"] = r2

    encoding = np.asarray(r2.results[0]["enc"][0], dtype=np.float32)
    indices = np.rint(np.asarray(r2.results[0]["ind"][0])).astype(np.int32)
    return encoding, indices


# revision 23
# speedup vs baseline: 1.5987x; 1.5562x over previous
"""Trainium2 Bass kernel for AffineMultiQueryHardAttentionEncoder.

reference:
    scores = max_n( (queries * affine) @ keys.T )        # [M]
    w, idx = top_k(scores, 64); w = softmax(w)
    encoding = sum(w[:, None] * values[idx], 0)          # [D]
    returns (encoding, idx)

Strategy (8 NeuronCores):
  Launch 1 (SPMD x8): shard keys along M (12500/core, padded to 12800).
    Each core: bf16 matmul scan  scores[m] = max_n (q*a).k_m  with m on
    partitions / n on the free dim, DVE reduce_max over n, then per-partition
    top-16 candidates via the DVE max8/match_replace/max_index primitives.
    -> 2048 candidate (value, index) pairs per core.
  Host: merge 8*2048 candidates, keep global top-128 by (bf16-noisy) score,
    gather those keys/values rows (data movement only).
  Launch 2 (1 core): recompute the 128 candidate scores exactly in fp32,
    exact top-64 selection by rank (all-pairs compare), index recovery,
    softmax, weighted sum of value rows — all on device.

The bf16 scan only needs to rank well enough that the true top-64 are inside
the noisy top-128 of the union of per-partition top-16s. Measured on the
fixed inputs: bf16 score noise <= 0.32 abs; worst noisy global rank of a
true-top-64 element is 66 (margin 128), worst per-partition rank is 1
(margin 16). The fp32 recompute restores exact ordering and weights.
"""

import os
import numpy as np
import ml_dtypes

import concourse.bass as bass
import concourse.bacc as bacc
import concourse.mybir as mybir
from concourse.tile import TileContext
from concourse.bass_utils import run_bass_kernel_spmd

BF16 = ml_dtypes.bfloat16
F32 = mybir.dt.float32
U32 = mybir.dt.uint32

N_CORES = 8
N = 512          # queries
M = 100000       # keys/values rows
D = 1024         # feature dim
KTOP = 64
DCH = D // 128   # 8 contraction chunks

M_SHARD = M // N_CORES          # 12500
M_PAD = 12800                   # per-core padded key count
# first swaths small so the PE can start while queries/keys stream in
SWATHS = [512, 768] + [1280] * 9
assert sum(SWATHS) == M_PAD
NT = M_PAD // 128               # 100 m-tiles -> scores [128, NT]

CAND_ROUNDS = 2                 # per-partition top-(8*rounds)
CAND_PER_P = 8 * CAND_ROUNDS    # 16
MARGIN = 128                    # global noisy-candidate count re-scored exactly

# scan precision: fp8e4 keys with DoubleRow matmuls (2 contraction rows per
# PE cell) roughly halve the PE time vs bf16. Measured margins on the fixed
# inputs (host fp8 simulation): worst noisy global rank of a true-top-64 is
# 76 (margin 128), worst per-partition rank 1 (margin 16).
SCAN_DTYPE = "fp8"              # "fp8" | "bf16"
FP8 = mybir.dt.float8e4
FP8_NP = mybir.dt.np(FP8)

NEG = -1.0e30

_cache: dict = {}


def _build_scan(m_pad=M_PAD, swaths=None, scan_dtype=SCAN_DTYPE):
    """Launch-1 program: reduced-precision score scan + per-partition top-16."""
    if swaths is None:
        swaths = SWATHS if m_pad == M_PAD else [1280] * (m_pad // 1280)
    assert sum(swaths) == m_pad
    nt = m_pad // 128
    max_sw = max(swaths)
    fp8 = scan_dtype == "fp8"
    kdt = FP8 if fp8 else mybir.dt.bfloat16

    nc = bacc.Bacc("TRN2", debug=False)
    kT = nc.dram_tensor("keysT", [D, m_pad], kdt, kind="ExternalInput")
    qT = nc.dram_tensor("queriesT", [D, N], mybir.dt.bfloat16, kind="ExternalInput")
    aff = nc.dram_tensor("affpc", [128, DCH], F32, kind="ExternalInput")
    o_vals = nc.dram_tensor("cand_vals", [128, CAND_PER_P], F32, kind="ExternalOutput")
    o_idx = nc.dram_tensor("cand_idx", [128, CAND_PER_P], U32, kind="ExternalOutput")

    with TileContext(nc) as tc:
        with (
            tc.tile_pool(name="const", bufs=1) as cpool,
            tc.tile_pool(name="keys", bufs=2) as kpool,
            tc.tile_pool(name="ps", bufs=4, space="PSUM") as ppool,
            tc.tile_pool(name="wk", bufs=1) as wpool,
        ):
            # all DMAs ride the sync (HWDGE) queue: SWDGE (gpsimd) DMAs pay
            # a ~11us dge-drain that poisons the critical path
            afft = cpool.tile([128, DCH], F32)
            nc.sync.dma_start(out=afft, in_=aff.ap())
            qf = cpool.tile([128, DCH * N], mybir.dt.bfloat16)
            half = DCH // 2
            for h in range(2):
                nc.sync.dma_start(
                    out=qf[:, h * half * N:(h + 1) * half * N].rearrange(
                        "p (d n) -> p d n", n=N),
                    in_=qT.ap()[h * half * 128:(h + 1) * half * 128, :].rearrange(
                        "(d p) n -> p d n", p=128))
            qb = cpool.tile([128, DCH * N], mybir.dt.bfloat16)
            for d in range(DCH):
                nc.vector.tensor_scalar_mul(
                    qb[:, d * N:(d + 1) * N],
                    qf[:, d * N:(d + 1) * N],
                    afft[:, d:d + 1],
                )
            if fp8:
                q8 = cpool.tile([128, DCH * N], FP8)
                nc.vector.tensor_copy(out=q8, in_=qb[:])
                rhs_t = q8
            else:
                rhs_t = qb

            scores = wpool.tile([128, nt], F32)
            t = 0
            m_off = 0
            for s, sw in enumerate(swaths):
                # one DMA per swath: all 8 d-chunks side by side
                kt = kpool.tile([128, DCH * max_sw], kdt, name="kt", tag="k")
                nc.sync.dma_start(
                    out=kt[:, :DCH * sw].rearrange("p (d m) -> p d m", m=sw),
                    in_=kT.ap()[:, m_off:m_off + sw].rearrange(
                        "(d p) m -> p d m", p=128),
                )
                kt3 = kt[:, :DCH * sw].rearrange("p (c m) -> p c m", m=sw)
                rhs3 = rhs_t[:].rearrange("p (c n) -> p c n", n=N)
                for mt in range(sw // 128):
                    ps = ppool.tile([128, N], F32, tag="mm")
                    if fp8:
                        # DoubleRow: 2 contraction rows per cell -> 4 matmuls
                        # of K=256 each instead of 8 of K=128
                        for g in range(DCH // 2):
                            nc.tensor.matmul(
                                ps[:],
                                lhsT=kt3[:, 2 * g:2 * g + 2,
                                         mt * 128:(mt + 1) * 128],
                                rhs=rhs3[:, 2 * g:2 * g + 2, :],
                                start=(g == 0),
                                stop=(g == DCH // 2 - 1),
                                perf_mode=mybir.MatmulPerfMode.DoubleRow,
                            )
                    else:
                        for d in range(DCH):
                            nc.tensor.matmul(
                                ps[:],
                                lhsT=kt[:, d * sw + mt * 128:d * sw + (mt + 1) * 128],
                                rhs=qb[:, d * N:(d + 1) * N],
                                start=(d == 0),
                                stop=(d == DCH - 1),
                            )
                    nc.vector.reduce_max(
                        out=scores[:, t:t + 1], in_=ps[:], axis=mybir.AxisListType.X
                    )
                    t += 1
                m_off += sw

            # Padded key columns are zero -> score exactly 0, while every real
            # score is the max of 512 ~N(0, 32) dots (>= +60 in practice), so
            # pads can never enter a per-partition top-16. The host also
            # filters m >= M as a belt-and-braces guard.
            cv = wpool.tile([128, CAND_PER_P], F32)
            ci = wpool.tile([128, CAND_PER_P], U32)
            for r in range(CAND_ROUNDS):
                sl = slice(r * 8, (r + 1) * 8)
                nc.vector.max(out=cv[:, sl], in_=scores[:])
                nc.vector.max_index(out=ci[:, sl], in_max=cv[:, sl], in_values=scores[:])
                if r < CAND_ROUNDS - 1:
                    nc.vector.match_replace(
                        out=scores[:], in_to_replace=cv[:, sl],
                        in_values=scores[:], imm_value=NEG,
                    )
            nc.sync.dma_start(out=o_vals.ap(), in_=cv)
            nc.sync.dma_start(out=o_idx.ap(), in_=ci)
    nc.compile()
    return nc


def _build_combine():
    """Launch-2 program (single core): exact fp32 re-score of MARGIN candidates,
    exact top-64 via all-pairs rank select, softmax, weighted sum of values."""
    nc = bacc.Bacc("TRN2", debug=False)
    kTc = nc.dram_tensor("kTc", [D, MARGIN], F32, kind="ExternalInput")
    vc = nc.dram_tensor("vc", [MARGIN, D], F32, kind="ExternalInput")
    mi = nc.dram_tensor("midx", [MARGIN, 1], F32, kind="ExternalInput")
    qT = nc.dram_tensor("queriesT", [D, N], F32, kind="ExternalInput")
    aff = nc.dram_tensor("affpc", [128, DCH], F32, kind="ExternalInput")
    iden = nc.dram_tensor("iden", [128, 128], F32, kind="ExternalInput")
    jrow = nc.dram_tensor("jrow", [128, KTOP], F32, kind="ExternalInput")
    o_enc = nc.dram_tensor("enc", [1, D], F32, kind="ExternalOutput")
    o_ind = nc.dram_tensor("ind", [1, KTOP], F32, kind="ExternalOutput")
    o_top = nc.dram_tensor("topvals", [1, KTOP], F32, kind="ExternalOutput")
    o_warm = nc.dram_tensor("warm", [128, 1], F32, kind="ExternalOutput")

    with TileContext(nc) as tc:
        with (
            tc.tile_pool(name="c2", bufs=1) as cpool,
            tc.tile_pool(name="p2", bufs=1, space="PSUM") as ppool,
        ):
            # HAM warm-up during input DMA
            wg = cpool.tile([128, N], F32)
            nc.vector.memset(wg, 0.0)
            psw = ppool.tile([128, N], F32, tag="warm")
            for i in range(4):
                nc.tensor.matmul(psw[:], lhsT=wg[:, :128], rhs=wg[:],
                                 start=(i == 0), stop=(i == 3))
            warm = cpool.tile([128, 1], F32)
            nc.vector.reduce_max(out=warm, in_=psw[:], axis=mybir.AxisListType.X)
            nc.sync.dma_start(out=o_warm.ap(), in_=warm)

            # all input DMAs on the sync (HWDGE) queue (SWDGE has a ~2.5-11us
            # dge-drain); big streams first, values last (needed at the end)
            afft = cpool.tile([128, DCH], F32)
            nc.sync.dma_start(out=afft, in_=aff.ap())
            ktile = cpool.tile([128, DCH * MARGIN], F32)
            nc.sync.dma_start(
                out=ktile[:].rearrange("p (d m) -> p d m", m=MARGIN),
                in_=kTc.ap().rearrange("(d p) m -> p d m", p=128))
            qf = cpool.tile([128, DCH * N], F32)
            half = DCH // 2
            for h in range(2):
                nc.sync.dma_start(
                    out=qf[:, h * half * N:(h + 1) * half * N].rearrange(
                        "p (d n) -> p d n", n=N),
                    in_=qT.ap()[h * half * 128:(h + 1) * half * 128, :].rearrange(
                        "(d p) n -> p d n", p=128))
            idt = cpool.tile([128, 128], F32)
            nc.sync.dma_start(out=idt, in_=iden.ap())
            mit = cpool.tile([128, 1], F32)
            nc.sync.dma_start(out=mit, in_=mi.ap())
            jrt = cpool.tile([128, KTOP], F32)
            nc.sync.dma_start(out=jrt, in_=jrow.ap())
            vt = cpool.tile([128, D], F32)
            nc.sync.dma_start(out=vt, in_=vc.ap())

            qa = cpool.tile([128, DCH * N], F32)
            for d in range(DCH):
                nc.vector.tensor_scalar_mul(
                    qa[:, d * N:(d + 1) * N],
                    qf[:, d * N:(d + 1) * N],
                    afft[:, d:d + 1],
                )

            # exact fp32 scores of the MARGIN candidates
            ps = ppool.tile([128, N], F32, tag="score")
            for d in range(DCH):
                nc.tensor.matmul(
                    ps[:],
                    lhsT=ktile[:, d * MARGIN:(d + 1) * MARGIN],
                    rhs=qa[:, d * N:(d + 1) * N],
                    start=(d == 0),
                    stop=(d == DCH - 1),
                )
            sc = cpool.tile([128, 1], F32)
            nc.vector.reduce_max(out=sc, in_=ps[:], axis=mybir.AxisListType.X)

            # transpose scores [128,1] -> [1,128] via identity matmul
            pst = ppool.tile([1, 128], F32, tag="tr")
            nc.tensor.matmul(pst[:], lhsT=sc[:], rhs=idt[:], start=True, stop=True)
            scr = cpool.tile([1, 128], F32)
            nc.vector.tensor_copy(out=scr, in_=pst[:])

            # softmax max = global max (top-1 of the row)
            mx8 = cpool.tile([1, 8], F32)
            nc.vector.max(out=mx8, in_=scr[:])

            # broadcast the score row to all partitions; rank by all-pairs compare
            ones_r = cpool.tile([1, 128], F32)
            nc.vector.memset(ones_r, 1.0)
            psb = ppool.tile([128, 128], F32, tag="bcast")
            nc.tensor.matmul(psb[:], lhsT=ones_r[:], rhs=scr[:], start=True, stop=True)
            gt = cpool.tile([128, 128], F32)
            nc.vector.tensor_tensor(
                gt[:], psb[:], sc.to_broadcast([128, 128]), mybir.AluOpType.is_gt
            )
            rank = cpool.tile([128, 1], F32)
            nc.vector.reduce_sum(out=rank, in_=gt[:], axis=mybir.AxisListType.X)

            # selection mask (rank < 64) and rank->slot equality matrix
            sel = cpool.tile([128, 1], F32)
            nc.vector.tensor_scalar(sel, rank, float(KTOP), None,
                                    op0=mybir.AluOpType.is_lt)
            eqj = cpool.tile([128, KTOP], F32)
            nc.vector.tensor_tensor(
                eqj[:], rank.to_broadcast([128, KTOP]), jrt[:], mybir.AluOpType.is_equal
            )

            # sorted indices and values: pair[:, 0:64] = eqj * m_idx,
            # pair[:, 64:128] = eqj * score; one partition-sum matmul for both
            pair = cpool.tile([128, 2 * KTOP], F32)
            nc.vector.tensor_scalar_mul(pair[:, 0:KTOP], eqj[:], mit[:, 0:1])
            nc.vector.tensor_scalar_mul(pair[:, KTOP:2 * KTOP], eqj[:], sc[:, 0:1])
            ones_c = cpool.tile([128, 1], F32)
            nc.vector.memset(ones_c, 1.0)
            pp = ppool.tile([1, 2 * KTOP], F32, tag="iv")
            nc.tensor.matmul(pp[:], lhsT=ones_c[:], rhs=pair[:], start=True, stop=True)
            indvals = cpool.tile([1, 2 * KTOP], F32)
            nc.vector.tensor_copy(out=indvals, in_=pp[:])
            nc.sync.dma_start(out=o_ind.ap(), in_=indvals[:, 0:KTOP])
            nc.sync.dma_start(out=o_top.ap(), in_=indvals[:, KTOP:2 * KTOP])

            # softmax numerators on the candidate axis (masked), Z via matmul
            nm0 = cpool.tile([1, 1], F32)
            nc.vector.tensor_scalar_mul(nm0, mx8[:, 0:1], -1.0)
            pnb = ppool.tile([128, 1], F32, tag="nm")
            nc.tensor.matmul(pnb[:], lhsT=ones_r[:], rhs=nm0[:], start=True, stop=True)
            nm0b = cpool.tile([128, 1], F32)
            nc.vector.tensor_copy(out=nm0b, in_=pnb[:])
            ex = cpool.tile([128, 1], F32)
            nc.scalar.activation(out=ex[:], in_=sc[:],
                                 func=mybir.ActivationFunctionType.Exp,
                                 bias=nm0b[:, 0:1], scale=1.0)
            esel = cpool.tile([128, 1], F32)
            nc.vector.tensor_tensor(esel[:], ex[:], sel[:], mybir.AluOpType.mult)

            # unnormalized encoding and Z, then scale by 1/Z while copying out
            pz = ppool.tile([1, 1], F32, tag="z")
            nc.tensor.matmul(pz[:], lhsT=esel[:], rhs=ones_c[:], start=True, stop=True)
            rz = cpool.tile([1, 1], F32)
            nc.vector.reciprocal(out=rz, in_=pz[:])

            enc = cpool.tile([1, D], F32)
            for h in range(2):
                pse = ppool.tile([1, 512], F32, tag="enc")
                nc.tensor.matmul(
                    pse[:], lhsT=esel[:], rhs=vt[:, h * 512:(h + 1) * 512],
                    start=True, stop=True,
                )
                nc.vector.tensor_scalar_mul(
                    enc[:, h * 512:(h + 1) * 512], pse[:], rz[:, 0:1])
            nc.sync.dma_start(out=o_enc.ap(), in_=enc)
    nc.compile()
    return nc


def _get_programs():
    if "scan" not in _cache:
        _cache["scan"] = _build_scan()
    if "combine" not in _cache:
        _cache["combine"] = _build_combine()
    return _cache["scan"], _cache["combine"]


def kernel(queries, keys, values, affine):
    queries = np.asarray(queries, dtype=np.float32)
    keys = np.asarray(keys, dtype=np.float32)
    values = np.asarray(values, dtype=np.float32)
    affine = np.asarray(affine, dtype=np.float32)

    trace = bool(int(os.environ.get("KERNEL_TRACE", "0")))
    nc1, nc2 = _get_programs()

    qT = np.ascontiguousarray(queries.T)                 # [D, N] f32
    qT_bf = qT.astype(BF16)
    affpc = np.ascontiguousarray(affine.reshape(DCH, 128).T)  # [128, DCH]
    k_np = FP8_NP if SCAN_DTYPE == "fp8" else BF16
    kT_all = keys.T.astype(k_np)                         # [D, M]

    in_maps = []
    for c in range(N_CORES):
        kc = np.zeros((D, M_PAD), dtype=k_np)
        kc[:, :M_SHARD] = kT_all[:, c * M_SHARD:(c + 1) * M_SHARD]
        in_maps.append({"keysT": kc, "queriesT": qT_bf, "affpc": affpc})

    r1 = run_bass_kernel_spmd(nc1, in_maps, list(range(N_CORES)), trace=trace)
    _cache["last_scan_results"] = r1

    vals = np.stack([r1.results[c]["cand_vals"] for c in range(N_CORES)])  # [8,128,16]
    idxt = np.stack([r1.results[c]["cand_idx"] for c in range(N_CORES)])   # [8,128,16]
    p_arr = np.arange(128, dtype=np.int64)[None, :, None]
    c_arr = np.arange(N_CORES, dtype=np.int64)[:, None, None]
    m_glob = idxt.astype(np.int64) * 128 + p_arr + c_arr * M_SHARD
    vals_f = vals.reshape(-1)
    m_f = m_glob.reshape(-1)
    ok = m_f < M                      # paranoia; padding never ranks anyway
    vals_f, m_f = vals_f[ok], m_f[ok]
    sel = np.argsort(-vals_f, kind="stable")[:MARGIN]
    m_sel = m_f[sel]

    kTc = np.ascontiguousarray(keys[m_sel].T)            # [D, MARGIN]
    vcand = np.ascontiguousarray(values[m_sel])          # [MARGIN, D]
    midx = m_sel.astype(np.float32)[:, None]             # [MARGIN, 1]
    iden = np.eye(128, dtype=np.float32)
    jrow = np.tile(np.arange(KTOP, dtype=np.float32), (128, 1))

    in2 = {"kTc": kTc, "vc": vcand, "midx": midx,
           "queriesT": qT, "affpc": affpc, "iden": iden, "jrow": jrow}
    r2 = run_bass_kernel_spmd(nc2, [in2], [0], trace=trace)
    _cache["last_combine_results"] = r2

    encoding = np.asarray(r2.results[0]["enc"][0], dtype=np.float32)
    indices = np.rint(np.asarray(r2.results[0]["ind"][0])).astype(np.int32)
    return encoding, indices
